# revision 10
# baseline (speedup 1.0000x reference)
"""Trainium2 Bass kernel for causal GQA self-attention (B=2,S=2048,D=1024,H=16,HKV=4,HD=64).

Sharding: 8 cores = DP(2 over batch) x TP(4 over GQA groups).
Each core computes, for one batch element and one GQA group (4 q heads + 1 kv head),
the partial output  y_group @ Wo[:, group_cols].T  (row-sharded Wo).
Host sums the 4 TP partials per batch element.

v2 design (vs the 226us v1):
- ONE fused pipeline: attention block b starts as soon as projection chunk
  b//2 is done; all remaining PE work (later proj chunks, Wo, normalize
  broadcasts, v transposes) is queued as fine-grained filler units pumped
  between the per-iteration score/pv matmuls, so the PE never idles and can
  hold its high p-state (stall resets drop it 2.4GHz -> 1.2GHz).
- j-loop software pipelining: pv(j) is emitted after scores(j+1), so the PE
  does not sit on the exp(j) semaphore (ACT paces at ~1.15us/iter).
- rms factors via f = exp(-0.5*ln(ssq/HD+eps)) so the WHOLE kernel uses the
  single natural_log_exp_and_others ACT table set (no 2.7us table reloads
  interleaved with the softmax exps).
- denominator reciprocal computed in a [128,8] layout (DMA-shifted) instead
  of [1,1024]: 8 DVE cycles instead of 1024.
- yt accumulators single-buffered: drained to SBUF bf16 right after the last
  pv, normalize runs later off SBUF; frees 2 PSUM banks for the proj ring.
- PSUM: big ring 2x[128,1024] (scores/Wo/broadcast/transpose), proj ring
  2x[128,512], yt01/yt23 1 bank each = 8 banks exactly.
- custom-DVE ops (reciprocal_approx_fast) have untracked reads/writes: every
  cross-engine edge goes through a tracked same-engine tensor_copy sentry.
- DMA issue stays on the hardware DGE queues (sync/scalar).
"""

import sys
from collections import deque
from contextlib import ExitStack

sys.path.insert(0, "/opt/trn_rl_repo")

import numpy as np
import ml_dtypes

import concourse.bass as bass
import concourse.bacc as bacc
import concourse.tile as tile
import concourse.mybir as mybir
from concourse.bass_utils import run_bass_kernel_spmd

BF16 = mybir.dt.bfloat16
F32 = mybir.dt.float32
F16 = mybir.dt.float16
AF = mybir.ActivationFunctionType
BF16NP = ml_dtypes.bfloat16

import os
KDEBUG = int(os.environ.get("KDEBUG", "0"))

D, H, HKV, HD, B, S = 1024, 16, 4, 64, 2, 2048
HG = 4              # q heads per core
KV_DIM = HKV * HD   # 256
E = HG * HD         # 256 local q-proj dim
ROPE_BASE = 10000.0
EPS = float(np.finfo(np.float32).eps)
MASK_NEG = -50.0

NK = D // 128       # 8 contraction tiles for qkv projections
SQB = 256           # sq block size in attention
NB = S // SQB       # 8 blocks
NJ = S // 128       # 16 sk tiles


def _consts():
    """Constant tensors baked into the NEFF (same for every core)."""
    i = np.arange(32, dtype=np.float64)
    inv_freq = 1.0 / (ROPE_BASE ** (2.0 * i / HD))
    pos = np.arange(S, dtype=np.float64)
    fr = pos[:, None] * inv_freq[None, :]           # [S, 32]
    cosT = np.cos(fr).T.astype(BF16NP)              # [32, S]
    sinT = np.sin(fr).T.astype(BF16NP)

    # mask bias for diagonal sk-tiles: pattern p in {0,1}
    # valid iff c >= 128*p + r   (r: sk row 0..127, c: sq col 0..255)
    r = np.arange(128)[:, None]
    c = np.arange(SQB)[None, :]
    mbs = []
    for p in range(2):
        m = np.where(c >= 128 * p + r, 0.0, MASK_NEG).astype(BF16NP)  # [128, 256]
        mbs.append(np.tile(m, (1, 2)))               # [128, 512] (2 head slots)

    sel36 = np.zeros((128, 36), dtype=BF16NP)        # q sumsq head selector
    for h in range(4):
        sel36[32 * h:32 * h + 32, h] = 1.0
    id128 = np.eye(128, dtype=BF16NP)
    return cosT, sinT, mbs, sel36, id128


def _build():
    nc = bacc.Bacc("TRN2", debug=False)

    xT_d = nc.dram_tensor("xT", [D, S], BF16, kind="ExternalInput")
    wq_d = nc.dram_tensor("wq", [128, NK, E], BF16, kind="ExternalInput")
    wkv_d = nc.dram_tensor("wkv", [128, NK, 128], BF16, kind="ExternalInput")
    wo_d = nc.dram_tensor("wo", [128, 2, D], BF16, kind="ExternalInput")
    gsel_d = nc.dram_tensor("gsel", [4, 128], BF16, kind="ExternalInput")
    out_d = nc.dram_tensor("out", [S, D], F16, kind="ExternalOutput")
    dbg = {}
    if KDEBUG:
        for nm, shp in [("d_qsb0", [128, S]), ("d_qsb1", [128, S]),
                        ("d_kvsb", [128, S]), ("d_kdup", [128, S]),
                        ("d_vsb", [128, NJ, 66]),
                        ("d_yn0", [128, S]), ("d_yn1", [128, S]),
                        ("d_ysb", [64, 1024]), ("d_pbs", [128, 512])]:
            dbg[nm] = nc.dram_tensor(nm, shp, BF16, kind="ExternalOutput")
        dbg["d_rbc"] = nc.dram_tensor("d_rbc", [1, 1024], F16, kind="ExternalOutput")
        dbg["d_ft"] = nc.dram_tensor("d_ft", [33, S], BF16, kind="ExternalOutput")

    cosT, sinT, mbs, sel36, id128 = _consts()
    cs_d = nc.inline_tensor(np.concatenate([cosT, sinT], axis=1), "cs")  # [32,2S]
    mb_d = nc.inline_tensor(np.concatenate(mbs, axis=1), "mb")           # [128,1024]
    sel36_d = nc.inline_tensor(sel36, "sel36")
    id128_d = nc.inline_tensor(id128, "id128")

    with tile.TileContext(nc) as tc, ExitStack() as ctx:
        sp = ctx.enter_context(tc.tile_pool(name="static", bufs=1))

        def stile(shape, dt, tag):
            return sp.tile(shape, dt, name=tag, tag=tag)

        # ---- static SBUF tensors ----
        xt_all = stile([128, NK, S], BF16, "xt")
        wq = stile([128, NK, E], BF16, "wq")
        wkv = stile([128, NK, 128], BF16, "wkv")
        wo = stile([128, 2, D], BF16, "wo")
        cs = stile([128, 2 * S], BF16, "cs")          # [cos | sin]
        mbt = stile([128, 1024], BF16, "mbt")         # [maskbias p0 | p1]
        sel36_s = stile([128, 36], BF16, "sel36")
        id128_s = stile([128, 128], BF16, "id128")
        gsel_s = stile([4, 128], BF16, "gsel")
        onesr = stile([128, 64], BF16, "onesr")      # bf16 ones
        onesf16 = stile([128, 64], F16, "onesf16")   # f16 ones (denom bcast lhsT)
        e8b = stile([128, 1], F32, "e8b")            # exp bias (0; kept as AP)
        epsb = stile([128, 1], F32, "epsb")          # eps bias AP for Ln

        qsb = [stile([128, S], BF16, f"qsb{m}") for m in range(2)]   # T/B packed
        kvsb = stile([128, S], BF16, "kvsb")          # k(0:64) | v(64:128)
        kb0 = stile([32, S], BF16, "kb0")             # k bottom half at partition 0
        qstd = [stile([128, S], BF16, f"qstd{m}") for m in range(2)]  # [h0;h1],[h2;h3]
        kdup = stile([128, S], BF16, "kdup")          # [k ; k] for both row groups
        vsb = stile([128, NJ, 66], BF16, "vsb")       # [v(0:64) | ones(64) | pad]
        ynA = stile([128, 2, S], BF16, "yn")          # normalized y^T, both halves
        pbs = stile([64, 1024], BF16, "pbs")          # bcast recip per block
        # cols 0:512 = (h0,h2) recips, 512:1024 = (h1,h3); base partition 0
        # so the DVE muls below have equal SBUF base partitions

        # ---- pools ----
        pbig = ctx.enter_context(
            tc.tile_pool(name="pbig", bufs=2, space=bass.MemorySpace.PSUM))
        pprj = ctx.enter_context(
            tc.tile_pool(name="pprj", bufs=2, space=bass.MemorySpace.PSUM))
        py01 = ctx.enter_context(
            tc.tile_pool(name="py01", bufs=1, space=bass.MemorySpace.PSUM))
        py23 = ctx.enter_context(
            tc.tile_pool(name="py23", bufs=1, space=bass.MemorySpace.PSUM))
        pa = ctx.enter_context(tc.tile_pool(name="pa", bufs=4))
        lns = ctx.enter_context(tc.tile_pool(name="lns", bufs=2))
        rt = ctx.enter_context(tc.tile_pool(name="rt", bufs=2))
        sst = ctx.enter_context(tc.tile_pool(name="sst", bufs=2))
        ob = ctx.enter_context(tc.tile_pool(name="ob", bufs=2))
        yb = ctx.enter_context(tc.tile_pool(name="yb", bufs=2))

        # ---- const / weight loads: k-interleaved so proj(0) starts ASAP ----
        for k in range(NK):
            nc.sync.dma_start(wq[:, k, :], wq_d[:, k, :])
            nc.sync.dma_start(wkv[:, k, :], wkv_d[:, k, :])
            nc.sync.dma_start(xt_all[:, k, 0:256], xT_d[128 * k:128 * (k + 1), 0:256])
        nc.scalar.dma_start(cs[0:32, :], cs_d[:])
        nc.scalar.dma_start(cs[32:64, :], cs[0:32, :])
        nc.scalar.dma_start(cs[64:128, :], cs[0:64, :])
        nc.scalar.dma_start(sel36_s[:], sel36_d[:])
        nc.scalar.dma_start(id128_s[:], id128_d[:])
        nc.scalar.dma_start(gsel_s[:], gsel_d[:])
        nc.scalar.dma_start(mbt[:], mb_d[:])
        for k in range(NK):
            nc.sync.dma_start(xt_all[:, k, 256:512], xT_d[128 * k:128 * (k + 1), 256:512])
        nc.scalar.dma_start(wo[:], wo_d[:])
        for k in range(NK):
            nc.sync.dma_start(xt_all[:, k, 512:1024], xT_d[128 * k:128 * (k + 1), 512:1024])
        for k in range(NK):
            nc.sync.dma_start(xt_all[:, k, 1024:S], xT_d[128 * k:128 * (k + 1), 1024:S])
        nc.vector.memset(onesr[:], 1.0)
        nc.vector.memset(onesf16[:], 1.0)
        nc.vector.memset(e8b[:], 0.0)
        nc.vector.memset(epsb[:], EPS)
        nc.vector.memset(vsb[:], 1.0)  # ones column at [:, j, 64]; 0:64 overwritten

        # ================= filler machinery =================
        fq = deque()
        ready = [False] * NB   # ready[b]: qstd/kdup/vsb cover block b's needs

        def pump(n=1):
            for _ in range(n):
                if fq:
                    fq.popleft()()

        def pump_until_ready(b):
            while not ready[b]:
                assert fq, f"filler queue empty but block {b} not ready"
                fq.popleft()()

        # ---- projection + rms/rope for a column range [c0, c1) ----
        def chunk_units(c0, c1, rdy_blocks):
            """Returns list of closures. Emitting all of them produces
            qstd/kdup/vsb for columns [c0, c1)."""
            w = c1 - c0
            sl = slice(c0, c1)
            slc = sl
            sls = slice(S + c0, S + c1)
            st = {}
            units = []

            def u_pq0a():
                st['pq0'] = pprj.tile([128, w], F32, name="pq0", tag="prj")
                for k in range(4):
                    nc.tensor.matmul(st['pq0'][:], wq[:, k, 0:128], xt_all[:, k, sl],
                                     start=(k == 0), stop=False)

            def u_pq0b():
                for k in range(4, NK):
                    nc.tensor.matmul(st['pq0'][:], wq[:, k, 0:128], xt_all[:, k, sl],
                                     start=False, stop=(k == NK - 1))
                nc.vector.tensor_copy(qsb[0][:, sl], st['pq0'][:])

            def u_pq1a():
                st['pq1'] = pprj.tile([128, w], F32, name="pq1", tag="prj")
                for k in range(4):
                    nc.tensor.matmul(st['pq1'][:], wq[:, k, 128:256], xt_all[:, k, sl],
                                     start=(k == 0), stop=False)

            def u_pq1b():
                for k in range(4, NK):
                    nc.tensor.matmul(st['pq1'][:], wq[:, k, 128:256], xt_all[:, k, sl],
                                     start=False, stop=(k == NK - 1))
                nc.vector.tensor_copy(qsb[1][:, sl], st['pq1'][:])

            def u_pkva():
                st['pkv'] = pprj.tile([128, w], F32, name="pkv", tag="prj")
                for k in range(4):
                    nc.tensor.matmul(st['pkv'][:], wkv[:, k, :], xt_all[:, k, sl],
                                     start=(k == 0), stop=False)

            def u_pkvb():
                for k in range(4, NK):
                    nc.tensor.matmul(st['pkv'][:], wkv[:, k, :], xt_all[:, k, sl],
                                     start=False, stop=(k == NK - 1))
                nc.scalar.copy(kvsb[:, sl], st['pkv'][:])
                nc.sync.dma_start(kb0[:, sl], kvsb[32:64, sl])

            def u_ssq():
                # squared sums -> per-head sums (rows 0:4 q heads, row 32 k)
                sq0 = rt.tile([128, w], BF16, name="sq0", tag="sq0")
                sq1 = rt.tile([128, w], BF16, name="sq1", tag="sq1")
                sqk = rt.tile([64, w], BF16, name="sqk", tag="sqk")
                nc.vector.tensor_mul(sq0[:], qsb[0][:, sl], qsb[0][:, sl])
                nc.vector.tensor_mul(sq1[:], qsb[1][:, sl], qsb[1][:, sl])
                nc.vector.tensor_mul(sqk[:], kvsb[0:64, sl], kvsb[0:64, sl])
                psqk = pprj.tile([36, w], F32, name="psqk", tag="prj")
                st['psqk'] = psqk
                nc.tensor.matmul(psqk[:], sel36_s[:], sq0[:], start=True, stop=False)
                nc.tensor.matmul(psqk[:], sel36_s[:], sq1[:], start=False, stop=True)
                nc.tensor.matmul(psqk[32:33, :], onesr[0:64, 0:1], sqk[:],
                                 start=False, stop=True, skip_group_check=True)

            def u_fact():
                # f = rsqrt(ssq/HD + eps) = exp(-0.5 * ln(ssq/HD + eps))
                # (keeps the whole kernel on the natural_log_exp table set)
                lnm = lns.tile([33, w], F32, name="lnm", tag="lnm")
                nc.scalar.activation(lnm[:], st['psqk'][0:33, :], AF.Ln,
                                     bias=epsb[0:33, :], scale=1.0 / HD)
                ft = lns.tile([33, w], BF16, name="ft", tag="ft")
                nc.scalar.activation(ft[:], lnm[:], AF.Exp, scale=-0.5)
                st['ft'] = ft
                if KDEBUG:
                    nc.sync.dma_start(dbg["d_ft"][:, sl], ft[:])
                # broadcast factors along hd rows via PE (gain/8 folded in gsel)
                fbq_ps = pprj.tile([128, w], F32, name="fbq", tag="prj")
                nc.tensor.matmul(fbq_ps[:], gsel_s[:], ft[0:4, :], start=True, stop=True)
                fbk_ps = pprj.tile([64, w], F32, name="fbk", tag="prj")
                nc.tensor.matmul(fbk_ps[:], onesr[32:33, 0:64], ft[32:33, :],
                                 start=True, stop=True, skip_group_check=True)
                fbq = lns.tile([128, w], BF16, name="fbq_s", tag="fbq_s")
                fbk = lns.tile([64, w], BF16, name="fbk_s", tag="fbk_s")
                nc.vector.tensor_copy(fbq[:], fbq_ps[:])
                nc.vector.tensor_copy(fbk[:], fbk_ps[:])
                st['fbq'], st['fbk'] = fbq, fbk

            def u_ropeq():
                fbq = st['fbq']
                t1 = rt.tile([128, w], BF16, name="t1", tag="t1")
                t2 = rt.tile([128, w], BF16, name="t2", tag="t2")
                qr0 = rt.tile([128, w], BF16, name="qr0", tag="qr0")
                qr1 = rt.tile([128, w], BF16, name="qr1", tag="qr1")
                nc.vector.tensor_mul(t1[:], qsb[0][:, sl], cs[:, slc])
                nc.vector.tensor_mul(t2[:], qsb[1][:, sl], cs[:, sls])
                nc.vector.tensor_add(t1[:], t1[:], t2[:])
                nc.vector.tensor_mul(qr0[:], t1[:], fbq[:])
                u1 = rt.tile([128, w], BF16, name="u1", tag="u1")
                u2 = rt.tile([128, w], BF16, name="u2", tag="u2")
                nc.vector.tensor_mul(u1[:], qsb[1][:, sl], cs[:, slc])
                nc.vector.tensor_mul(u2[:], qsb[0][:, sl], cs[:, sls])
                nc.vector.tensor_sub(u1[:], u1[:], u2[:])
                nc.vector.tensor_mul(qr1[:], u1[:], fbq[:])
                # reassemble per-head layout (DMA partition moves)
                for h in range(4):
                    dst = qstd[h // 2]
                    base = 64 * (h % 2)
                    hs = slice(32 * h, 32 * h + 32)
                    nc.sync.dma_start(dst[base:base + 32, sl], qr0[hs, :])
                    nc.sync.dma_start(dst[base + 32:base + 64, sl], qr1[hs, :])

            def u_ropek():
                fbk = st['fbk']
                k1 = rt.tile([32, w], BF16, name="k1", tag="k1")
                k2 = rt.tile([32, w], BF16, name="k2", tag="k2")
                kw0 = rt.tile([32, w], BF16, name="kw0", tag="kw0")
                kw1 = rt.tile([32, w], BF16, name="kw1", tag="kw1")
                nc.vector.tensor_mul(k1[:], kvsb[0:32, sl], cs[0:32, slc])
                nc.vector.tensor_mul(k2[:], kb0[:, sl], cs[0:32, sls])
                nc.vector.tensor_add(k1[:], k1[:], k2[:])
                nc.vector.tensor_mul(kw0[:], k1[:], fbk[0:32, :])
                k3 = rt.tile([32, w], BF16, name="k3", tag="k3")
                k4 = rt.tile([32, w], BF16, name="k4", tag="k4")
                nc.vector.tensor_mul(k3[:], kb0[:, sl], cs[0:32, slc])
                nc.vector.tensor_mul(k4[:], kvsb[0:32, sl], cs[0:32, sls])
                nc.vector.tensor_sub(k3[:], k3[:], k4[:])
                nc.vector.tensor_mul(kw1[:], k3[:], fbk[0:32, :])
                nc.sync.dma_start(kdup[0:32, sl], kw0[:])
                nc.sync.dma_start(kdup[32:64, sl], kw1[:])
                nc.sync.dma_start(kdup[64:96, sl], kw0[:])
                nc.sync.dma_start(kdup[96:128, sl], kw1[:])

            def u_vtr():
                # v transpose via PE, batched into one big-ring slot
                ntr = w // 128
                j0 = c0 // 128
                ptr = pbig.tile([128, 64 * ntr], BF16, name="ptr", tag="big")
                for t in range(ntr):
                    stj = j0 + t
                    nc.tensor.transpose(
                        ptr[:, 64 * t:64 * (t + 1)],
                        kvsb[64:128, 128 * stj:128 * (stj + 1)],
                        id128_s[64:128, 64:128])
                nc.vector.tensor_copy(
                    vsb[:, j0:j0 + ntr, 0:64],
                    ptr[:].rearrange("p (t e) -> p t e", t=ntr))
                for b_ in rdy_blocks:
                    ready[b_] = True

            return [u_pq0a, u_pq0b, u_pq1a, u_pq1b, u_pkva, u_pkvb,
                    u_ssq, u_fact, u_ropeq, u_ropek, u_vtr]

        # ================= attention machinery =================
        blk = {}            # b -> (yt01, yt23)
        pend_norm = [None]  # (b, ysb, rbc)
        pt_tiles = {}       # (b, j) -> pt tile

        def emit_sc(b, j):
            sq = slice(SQB * b, SQB * (b + 1))
            jt = slice(128 * j, 128 * (j + 1))
            stl = pbig.tile([128, 1024], F32, name="stl", tag="big")
            diag = j - 2 * b
            if diag >= 0:
                mbsl = slice(512 * diag, 512 * (diag + 1))
                nc.tensor.matmul(stl[:, 0:512], id128_s[:], mbt[:, mbsl],
                                 start=True, stop=False, skip_group_check=True)
                nc.tensor.matmul(stl[:, 512:1024], id128_s[:], mbt[:, mbsl],
                                 start=True, stop=False, skip_group_check=True)
            sflag = diag < 0
            # cols: h0 0:256 | h2 256:512 | h1 512:768 | h3 768:1024
            nc.tensor.matmul(stl[:, 0:256], kdup[0:64, jt],
                             qstd[0][0:64, sq], start=sflag, stop=True,
                             skip_group_check=True)
            nc.tensor.matmul(stl[:, 512:768], kdup[64:128, jt],
                             qstd[0][64:128, sq], start=sflag, stop=True,
                             skip_group_check=True)
            nc.tensor.matmul(stl[:, 256:512], kdup[0:64, jt],
                             qstd[1][0:64, sq], start=sflag, stop=True,
                             skip_group_check=True)
            nc.tensor.matmul(stl[:, 768:1024], kdup[64:128, jt],
                             qstd[1][64:128, sq], start=sflag, stop=True,
                             skip_group_check=True)
            pt = pa.tile([128, 1024], BF16, name="pt", tag="pt")
            nc.scalar.activation(pt[:], stl[:], AF.Exp, bias=e8b[:, :])
            pt_tiles[(b, j)] = pt

        def emit_den_chain(b, y1, y2):
            """Denominator reciprocal + bf16 drain of yt, right after the
            final pv of block b.  recip runs in a [128,8] layout (DMA
            shifted) - 8 DVE cycles instead of 1024 - then shifts back to
            [1,1024] f16 for the PE broadcast."""
            dcp = sst.tile([65, 1024], F32, name="dcp", tag="dcp")
            nc.vector.tensor_copy(dcp[64:65, 0:512], y1[64:65, :])
            nc.vector.tensor_copy(dcp[64:65, 512:1024], y2[64:65, :])
            dlow = sst.tile([128, 8], F32, name="dlow", tag="dlow")
            # shape-mismatched DMA: both sides linearize row-major, so
            # [1,1024] -> [128,8] puts element i at (i//8, i%8)
            nc.sync.dma_start(dlow[:], dcp[64:65, 0:1024])
            rlow = sst.tile([128, 8], F32, name="rlow", tag="rlow")
            # tracked DVE read of dlow: forces the wait on the shift DMA
            # before the (untracked) custom-op read below
            nc.vector.tensor_copy(rlow[:, 0:1], dlow[:, 0:1])
            nc.vector.reciprocal_approx_fast(rlow[:], dlow[:])
            rb8 = sst.tile([128, 8], F16, name="rb8", tag="rb8")
            # tracked copy of the untracked recip output (same engine)
            nc.vector.tensor_copy(rb8[:], rlow[:])
            rbc = sst.tile([1, 1024], F16, name="rbc", tag="rbc")
            nc.sync.dma_start(rbc[:], rb8[:])
            # bf16 drain of the numerators (frees the single-buffered psum)
            ysb = yb.tile([64, 1024], BF16, name="ysb", tag="ysb")
            nc.vector.tensor_copy(ysb[:, 0:512], y1[0:64, :])
            nc.vector.tensor_copy(ysb[:, 512:1024], y2[0:64, :])
            pend_norm[0] = (b, ysb, rbc)

        def make_pv(b, j):
            jmax = 2 * b + 1

            def f():
                if j == 0:
                    yt01 = py01.tile([65, 512], F32, name="yt01", tag="yt01")
                    yt23 = py23.tile([65, 512], F32, name="yt23", tag="yt23")
                    blk[b] = (yt01, yt23)
                y1, y2 = blk[b]
                pt = pt_tiles.pop((b, j))
                nc.tensor.matmul(y1[:], vsb[:, j, 0:65], pt[:, 0:512],
                                 start=(j == 0), stop=(j == jmax),
                                 skip_group_check=True)
                nc.tensor.matmul(y2[:], vsb[:, j, 0:65], pt[:, 512:1024],
                                 start=(j == 0), stop=(j == jmax),
                                 skip_group_check=True)
                if j == jmax:
                    emit_den_chain(b, y1, y2)
            return f

        def finish_normalize():
            b, ysb, rbc = pend_norm[0]
            pend_norm[0] = None
            sq = slice(SQB * b, SQB * (b + 1))
            pbb = pbig.tile([64, 1024], F32, name="pbb", tag="big")
            nc.tensor.matmul(pbb[:, 0:512], onesf16[0:1, 0:64], rbc[0:1, 0:512],
                             start=True, stop=True, skip_group_check=True)
            nc.tensor.matmul(pbb[:, 512:1024], onesf16[0:1, 0:64], rbc[0:1, 512:1024],
                             start=True, stop=True, skip_group_check=True)
            nc.vector.tensor_copy(pbs[:], pbb[:])
            # h0 -> yn slot 0 rows 0:64, h2 -> slot 1 rows 0:64 (direct)
            nc.vector.tensor_mul(
                ynA[0:64, :, sq],
                ysb[:, 0:512].rearrange("p (u c) -> p u c", u=2),
                pbs[:, 0:512].rearrange("p (u c) -> p u c", u=2))
            # h1/h3 -> yn rows 64:128 (via shift DMA)
            sg1 = sst.tile([64, 2, 256], BF16, name="sg1", tag="sg1")
            nc.vector.tensor_mul(
                sg1[:],
                ysb[:, 512:1024].rearrange("p (u c) -> p u c", u=2),
                pbs[:, 512:1024].rearrange("p (u c) -> p u c", u=2))
            nc.sync.dma_start(ynA[64:128, :, sq], sg1[:])
            if KDEBUG and b == 0:
                nc.sync.dma_start(dbg["d_pbs"][:], pbs[:])
                nc.sync.dma_start(dbg["d_ysb"][:], ysb[:])
                nc.sync.dma_start(dbg["d_rbc"][:], rbc[:])
            return b

        def wo_units(b):
            """Output projection for sq block b, as 4 single-slot units."""
            units = []
            for t2 in range(2):
                sti = 2 * b + t2
                ssl = slice(128 * sti, 128 * (sti + 1))
                stt = {}

                def ua(ssl=ssl, stt=stt):
                    pot = pbig.tile([128, 1024], F32, name="pot", tag="big")
                    stt['pot'] = pot
                    nc.tensor.matmul(pot[:, 0:512], ynA[:, 0, ssl], wo[:, 0, 0:512],
                                     start=True, stop=False)
                    nc.tensor.matmul(pot[:, 0:512], ynA[:, 1, ssl], wo[:, 1, 0:512],
                                     start=False, stop=True)

                def ub(ssl=ssl, stt=stt):
                    pot = stt['pot']
                    ot = ob.tile([128, D], F16, name="ot", tag="ot")
                    stt['ot'] = ot
                    nc.tensor.matmul(pot[:, 512:1024], ynA[:, 0, ssl], wo[:, 0, 512:1024],
                                     start=True, stop=False)
                    nc.tensor.matmul(pot[:, 512:1024], ynA[:, 1, ssl], wo[:, 1, 512:1024],
                                     start=False, stop=True)
                    nc.scalar.copy(ot[:, 0:512], pot[:, 0:512])

                def uc(ssl=ssl, stt=stt):
                    pot = stt['pot']
                    ot = stt['ot']
                    nc.vector.tensor_copy(ot[:, 512:1024], pot[:, 512:1024])
                    nc.sync.dma_start(out_d[ssl, :], ot[:])

                units += [ua, ub, uc]
            return units

        # ================= main schedule =================
        # chunk 0 in two 256-col halves (emitted inline for fast start);
        # chunks 1-3 at 512 cols via the filler queue.
        for u in chunk_units(0, 256, [0]):
            u()
        for u in chunk_units(256, 512, [1]):
            fq.append(u)

        prev_pv = None
        for b in range(NB):
            pump_until_ready(b)
            if b == 0:
                for u in chunk_units(512, 1024, [2, 3]):
                    fq.append(u)
            elif b == 2:
                for u in chunk_units(1024, 1536, [4, 5]):
                    fq.append(u)
            elif b == 4:
                for u in chunk_units(1536, 2048, [6, 7]):
                    fq.append(u)
            jmax = 2 * b + 1
            for j in range(jmax + 1):
                emit_sc(b, j)
                if prev_pv is not None:
                    prev_pv()
                prev_pv = make_pv(b, j)
                if j == 1 and pend_norm[0] is not None:
                    nb_ = finish_normalize()
                    for u in wo_units(nb_):
                        fq.append(u)
                pump(1)
            # extra pump late in the schedule to drain the wo backlog
            if b >= 5:
                pump(2)

        # ---- tail ----
        prev_pv()                 # final pv + den chain for block 7
        while fq:
            fq.popleft()()
        finish_normalize()        # block 7
        for u in wo_units(NB - 1):
            u()
        if KDEBUG:
            nc.sync.dma_start(dbg["d_qsb0"][:], qsb[0][:])
            nc.sync.dma_start(dbg["d_qsb1"][:], qsb[1][:])
            nc.sync.dma_start(dbg["d_kvsb"][:], kvsb[:])
            nc.sync.dma_start(dbg["d_kdup"][:], kdup[:])
            nc.sync.dma_start(dbg["d_vsb"][:], vsb[:])
            nc.sync.dma_start(dbg["d_yn0"][:], ynA[:, 0, :])
            nc.sync.dma_start(dbg["d_yn1"][:], ynA[:, 1, :])

    nc.finalize()
    return nc


_NC = None


def _get_nc():
    global _NC
    if _NC is None:
        _NC = _build()
    return _NC


def _perm():
    tops = [h * 64 + i for h in range(HG) for i in range(32)]
    bots = [h * 64 + 32 + i for h in range(HG) for i in range(32)]
    return tops + bots


def build_inmaps(x, Wq, Wk, Wv, Wo, q_gain):
    x = np.asarray(x, dtype=np.float32)
    Wq = np.asarray(Wq, dtype=np.float32)
    Wk = np.asarray(Wk, dtype=np.float32)
    Wv = np.asarray(Wv, dtype=np.float32)
    Wo = np.asarray(Wo, dtype=np.float32)
    q_gain = np.asarray(q_gain, dtype=np.float32)

    perm = _perm()
    xTs = [np.ascontiguousarray(x[dp].T).astype(BF16NP) for dp in range(2)]
    tp_maps = []
    for tp in range(4):
        wq_sel = Wq[tp * E:(tp + 1) * E].T[:, perm]          # [D, 256] permuted
        wq_t = np.ascontiguousarray(
            wq_sel.astype(BF16NP).reshape(NK, 128, E).transpose(1, 0, 2))
        wk_sel = Wk[tp * HD:(tp + 1) * HD].T                  # [D, 64]
        wv_sel = Wv[tp * HD:(tp + 1) * HD].T
        wkv_t = np.concatenate([wk_sel, wv_sel], axis=1).astype(BF16NP)
        wkv_t = np.ascontiguousarray(
            wkv_t.reshape(NK, 128, 128).transpose(1, 0, 2))
        wo_sel = Wo[:, tp * E:(tp + 1) * E].T                 # [256, D]
        wo_t = np.ascontiguousarray(
            wo_sel.astype(BF16NP).reshape(2, 128, D).transpose(1, 0, 2))
        g = q_gain[tp * HG:(tp + 1) * HG].astype(np.float32)
        gsel = np.zeros((4, 128), dtype=BF16NP)
        for h in range(4):
            gsel[h, 32 * h:32 * h + 32] = BF16NP(g[h] / 8.0)
        tp_maps.append({"wq": wq_t, "wkv": wkv_t, "wo": wo_t, "gsel": gsel})
    in_maps = []
    for c in range(8):
        dp, tp = divmod(c, 4)
        m = dict(tp_maps[tp])
        m["xT"] = xTs[dp]
        in_maps.append(m)
    return in_maps


def kernel(x, Wq, Wk, Wv, Wo, q_gain):
    in_maps = build_inmaps(x, Wq, Wk, Wv, Wo, q_gain)
    nc = _get_nc()
    res = run_bass_kernel_spmd(nc, in_maps, core_ids=list(range(8)))
    out = np.zeros((B, S, D), dtype=np.float32)
    for c in range(8):
        out[c // 4] += res.results[c]["out"].astype(np.float32)
    return out


# revision 14
# speedup vs baseline: 1.1735x; 1.1735x over previous
"""Trainium2 Bass kernel for causal GQA self-attention (B=2,S=2048,D=1024,H=16,HKV=4,HD=64).

Sharding: 8 cores = DP(2 over batch) x TP(4 over GQA groups).
Each core computes, for one batch element and one GQA group (4 q heads + 1 kv head),
the partial output  y_group @ Wo[:, group_cols].T  (row-sharded Wo).
Host sums the 4 TP partials per batch element.

v3 design (PE is the bottleneck: measured PE column rate ~1.37 GHz fixed):
- ONE fused pipeline: attention block b starts as soon as projection chunk
  b//2 is done; later proj chunks, Wo, normalize broadcasts and v transposes
  are queued as fine-grained filler units pumped between the per-iteration
  score/pv matmuls, so the PE stays busy.
- j-loop software pipelining: pv(j) is emitted after scores(j+1).
- scores = 2 matmuls/iter (one per PE row-group) via [64, 2, S] q packing.
- causal mask written into the scores PSUM by the DVE one iteration ahead
  (slot pre-allocation), instead of PE identity-matmuls.
- rms factors via f = exp(-0.5*ln(ssq/HD+eps)): the WHOLE kernel uses the
  single natural_log_exp_and_others ACT table set (1 table load).
- denominator recip in a [128,8] layout (DMA-shifted via the scalar queue -
  the sync queue is kept for the latency-tolerant bulk of the DMAs).
- yt accumulators single-buffered, drained to SBUF bf16 right after the last
  pv; normalize runs later off SBUF; frees 2 PSUM banks for the proj ring.
- PSUM: big ring 2x[128,1024] (scores/Wo/broadcast/transpose), proj ring
  2x[128,512], yt01/yt23 1 bank each = 8 banks exactly.
- DMA issue cost (~650ns per descriptor on the issuing queue) dominates the
  ramp: weights load as single big DMAs, x in 8+8 per-k slices split across
  the sync (first 512 cols) and scalar (rest) queues.
- custom-DVE ops (reciprocal_approx_fast) have untracked reads/writes: every
  cross-engine edge goes through a tracked same-engine tensor_copy sentry.
"""

import sys
from collections import deque
from contextlib import ExitStack

sys.path.insert(0, "/opt/trn_rl_repo")

import numpy as np
import ml_dtypes

import concourse.bass as bass
import concourse.bacc as bacc
import concourse.tile as tile
import concourse.mybir as mybir
from concourse.bass_utils import run_bass_kernel_spmd

BF16 = mybir.dt.bfloat16
F32 = mybir.dt.float32
F16 = mybir.dt.float16
AF = mybir.ActivationFunctionType
BF16NP = ml_dtypes.bfloat16

import os
KDEBUG = int(os.environ.get("KDEBUG", "0"))

D, H, HKV, HD, B, S = 1024, 16, 4, 64, 2, 2048
HG = 4              # q heads per core
KV_DIM = HKV * HD   # 256
E = HG * HD         # 256 local q-proj dim
ROPE_BASE = 10000.0
EPS = float(np.finfo(np.float32).eps)
MASK_NEG = -50.0

NK = D // 128       # 8 contraction tiles for qkv projections
SQB = 256           # sq block size in attention
NB = S // SQB       # 8 blocks
NJ = S // 128       # 16 sk tiles


def _consts():
    """Constant tensors baked into the NEFF (same for every core)."""
    i = np.arange(32, dtype=np.float64)
    inv_freq = 1.0 / (ROPE_BASE ** (2.0 * i / HD))
    pos = np.arange(S, dtype=np.float64)
    fr = pos[:, None] * inv_freq[None, :]           # [S, 32]
    cosT = np.cos(fr).T.astype(BF16NP)              # [32, S]
    sinT = np.sin(fr).T.astype(BF16NP)

    # mask bias for diagonal sk-tiles: pattern p in {0,1}
    # valid iff c >= 128*p + r   (r: sk row 0..127, c: sq col 0..255)
    r = np.arange(128)[:, None]
    c = np.arange(SQB)[None, :]
    mbs = []
    for p in range(2):
        m = np.where(c >= 128 * p + r, 0.0, MASK_NEG).astype(BF16NP)  # [128, 256]
        mbs.append(np.tile(m, (1, 2)))               # [128, 512] (2 head slots)

    sel36 = np.zeros((128, 36), dtype=BF16NP)        # q sumsq head selector
    for h in range(4):
        sel36[32 * h:32 * h + 32, h] = 1.0
    id128 = np.eye(128, dtype=BF16NP)
    return cosT, sinT, mbs, sel36, id128


def _build():
    nc = bacc.Bacc("TRN2", debug=False)

    xT_d = nc.dram_tensor("xT", [D, S], BF16, kind="ExternalInput")
    wq_d = nc.dram_tensor("wq", [128, NK, E], BF16, kind="ExternalInput")
    wkv_d = nc.dram_tensor("wkv", [128, NK, 128], BF16, kind="ExternalInput")
    wo_d = nc.dram_tensor("wo", [128, 2, D], BF16, kind="ExternalInput")
    gsel_d = nc.dram_tensor("gsel", [4, 128], BF16, kind="ExternalInput")
    out_d = nc.dram_tensor("out", [S, D], F16, kind="ExternalOutput")
    dbg = {}
    if KDEBUG:
        for nm, shp in [("d_qsb0", [128, S]), ("d_qsb1", [128, S]),
                        ("d_kvsb", [128, S]), ("d_kdup", [128, S]),
                        ("d_vsb", [128, NJ, 66]),
                        ("d_yn0", [128, S]), ("d_yn1", [128, S]),
                        ("d_ysb", [64, 1024]), ("d_pbs", [64, 1024])]:
            dbg[nm] = nc.dram_tensor(nm, shp, BF16, kind="ExternalOutput")
        dbg["d_rbc"] = nc.dram_tensor("d_rbc", [1, 1024], F16, kind="ExternalOutput")
        dbg["d_ft"] = nc.dram_tensor("d_ft", [33, S], BF16, kind="ExternalOutput")

    cosT, sinT, mbs, sel36, id128 = _consts()
    cs_d = nc.inline_tensor(np.concatenate([cosT, sinT], axis=1), "cs")  # [32,2S]
    mb_d = nc.inline_tensor(np.concatenate(mbs, axis=1), "mb")           # [128,1024]
    sel36_d = nc.inline_tensor(sel36, "sel36")
    id128_d = nc.inline_tensor(id128, "id128")

    with tile.TileContext(nc) as tc, ExitStack() as ctx:
        sp = ctx.enter_context(tc.tile_pool(name="static", bufs=1))

        def stile(shape, dt, tag):
            return sp.tile(shape, dt, name=tag, tag=tag)

        # ---- static SBUF tensors ----
        xt_all = stile([128, NK, S], BF16, "xt")
        wq = stile([128, NK, E], BF16, "wq")
        wkv = stile([128, NK, 128], BF16, "wkv")
        wo = stile([128, 2, D], BF16, "wo")
        cs = stile([128, 2 * S], BF16, "cs")          # [cos | sin]
        mbt = stile([128, 1024], BF16, "mbt")         # [maskbias p0 | p1]
        sel36_s = stile([128, 36], BF16, "sel36")
        id128_s = stile([128, 128], BF16, "id128")
        gsel_s = stile([4, 128], BF16, "gsel")
        onesr = stile([128, 64], BF16, "onesr")      # bf16 ones
        onesf16 = stile([128, 64], F16, "onesf16")   # f16 ones (denom bcast lhsT)
        e8b = stile([128, 1], F32, "e8b")            # exp bias (0; kept as AP)
        epsb = stile([128, 1], F32, "epsb")          # eps bias AP for Ln

        qsb = [stile([128, S], BF16, f"qsb{m}") for m in range(2)]   # T/B packed
        kvsb = stile([128, S], BF16, "kvsb")          # k(0:64) | v(64:128)
        kb0 = stile([32, S], BF16, "kb0")             # k bottom half at partition 0
        # qp[0]: rows 0:64  = [h0 | h2] interleaved per (u, sq); pairs with kdup[0:64]
        # qp[1]: rows 64:128 = [h1 | h3]; pairs with kdup[64:128]
        qp = [stile([128, 2, S], BF16, f"qp{m}") for m in range(2)]
        kdup = stile([128, S], BF16, "kdup")          # [k ; k] for both row groups
        vsb = stile([128, NJ, 66], BF16, "vsb")       # [v(0:64) | ones(64) | pad]
        ynA = stile([128, 2, S], BF16, "yn")          # normalized y^T, both halves
        pbs = stile([64, 1024], BF16, "pbs")          # bcast recip per block

        # ---- pools ----
        pbig = ctx.enter_context(
            tc.tile_pool(name="pbig", bufs=2, space=bass.MemorySpace.PSUM))
        pprj = ctx.enter_context(
            tc.tile_pool(name="pprj", bufs=2, space=bass.MemorySpace.PSUM))
        py01 = ctx.enter_context(
            tc.tile_pool(name="py01", bufs=1, space=bass.MemorySpace.PSUM))
        py23 = ctx.enter_context(
            tc.tile_pool(name="py23", bufs=1, space=bass.MemorySpace.PSUM))
        pa = ctx.enter_context(tc.tile_pool(name="pa", bufs=4))
        lns = ctx.enter_context(tc.tile_pool(name="lns", bufs=2))
        rt = ctx.enter_context(tc.tile_pool(name="rt", bufs=2))
        sst = ctx.enter_context(tc.tile_pool(name="sst", bufs=2))
        ob = ctx.enter_context(tc.tile_pool(name="ob", bufs=2))
        yb = ctx.enter_context(tc.tile_pool(name="yb", bufs=2))

        # ---- loads: few big DMAs; sync = latency-critical, scalar = bulk ----
        nc.sync.dma_start(wq[:], wq_d[:])
        nc.sync.dma_start(wkv[:], wkv_d[:])
        for k in range(NK):
            nc.sync.dma_start(xt_all[:, k, 0:512], xT_d[128 * k:128 * (k + 1), 0:512])
        nc.sync.dma_start(gsel_s[:], gsel_d[:])
        nc.scalar.dma_start(cs[0:32, :], cs_d[:])
        nc.scalar.dma_start(cs[32:64, :], cs[0:32, :])
        nc.scalar.dma_start(cs[64:128, :], cs[0:64, :])
        nc.scalar.dma_start(sel36_s[:], sel36_d[:])
        nc.scalar.dma_start(id128_s[:], id128_d[:])
        nc.scalar.dma_start(mbt[:], mb_d[:])
        for k in range(NK):
            nc.scalar.dma_start(xt_all[:, k, 512:S], xT_d[128 * k:128 * (k + 1), 512:S])
        nc.scalar.dma_start(wo[:], wo_d[:])
        nc.vector.memset(onesr[:], 1.0)
        nc.vector.memset(onesf16[:], 1.0)
        nc.vector.memset(e8b[:], 0.0)
        nc.vector.memset(epsb[:], EPS)
        nc.vector.memset(vsb[:], 1.0)  # ones column at [:, j, 64]; 0:64 overwritten

        # ================= filler machinery =================
        fq_hi = deque()   # chunk units (have deadlines)
        fq_lo = deque()   # wo units (latency-tolerant)
        ready = [False] * NB   # ready[b]: qp/kdup/vsb cover block b's needs

        def pump(n=1):
            for _ in range(n):
                if fq_hi:
                    fq_hi.popleft()()
                elif fq_lo:
                    fq_lo.popleft()()

        def pump_until_ready(b):
            while not ready[b]:
                assert fq_hi, f"hi queue empty but block {b} not ready"
                fq_hi.popleft()()

        # ---- projection + rms/rope units for a 512-column chunk ----
        def chunk_units(c0, rdy_blocks):
            w = 512
            c1 = c0 + w
            sl = slice(c0, c1)
            sls = slice(S + c0, S + c1)
            st = {}

            def u_pq0a():
                st['pq0'] = pprj.tile([128, w], F32, name="pq0", tag="prj")
                for k in range(4):
                    nc.tensor.matmul(st['pq0'][:], wq[:, k, 0:128], xt_all[:, k, sl],
                                     start=(k == 0), stop=False)

            def u_pq0b():
                for k in range(4, NK):
                    nc.tensor.matmul(st['pq0'][:], wq[:, k, 0:128], xt_all[:, k, sl],
                                     start=False, stop=(k == NK - 1))
                nc.vector.tensor_copy(qsb[0][:, sl], st['pq0'][:])

            def u_pq1a():
                st['pq1'] = pprj.tile([128, w], F32, name="pq1", tag="prj")
                for k in range(4):
                    nc.tensor.matmul(st['pq1'][:], wq[:, k, 128:256], xt_all[:, k, sl],
                                     start=(k == 0), stop=False)

            def u_pq1b():
                for k in range(4, NK):
                    nc.tensor.matmul(st['pq1'][:], wq[:, k, 128:256], xt_all[:, k, sl],
                                     start=False, stop=(k == NK - 1))
                nc.vector.tensor_copy(qsb[1][:, sl], st['pq1'][:])

            def u_pkva():
                st['pkv'] = pprj.tile([128, w], F32, name="pkv", tag="prj")
                for k in range(4):
                    nc.tensor.matmul(st['pkv'][:], wkv[:, k, :], xt_all[:, k, sl],
                                     start=(k == 0), stop=False)

            def u_pkvb():
                for k in range(4, NK):
                    nc.tensor.matmul(st['pkv'][:], wkv[:, k, :], xt_all[:, k, sl],
                                     start=False, stop=(k == NK - 1))
                nc.vector.tensor_copy(kvsb[:, sl], st['pkv'][:])
                nc.sync.dma_start(kb0[:, sl], kvsb[32:64, sl])

            def u_ssq():
                sq0 = rt.tile([128, w], BF16, name="sq0", tag="sq0")
                sq1 = rt.tile([128, w], BF16, name="sq1", tag="sq1")
                sqk = rt.tile([64, w], BF16, name="sqk", tag="sqk")
                nc.vector.tensor_mul(sq0[:], qsb[0][:, sl], qsb[0][:, sl])
                nc.vector.tensor_mul(sq1[:], qsb[1][:, sl], qsb[1][:, sl])
                nc.vector.tensor_mul(sqk[:], kvsb[0:64, sl], kvsb[0:64, sl])
                psqk = pprj.tile([36, w], F32, name="psqk", tag="prj")
                st['psqk'] = psqk
                nc.tensor.matmul(psqk[:], sel36_s[:], sq0[:], start=True, stop=False)
                nc.tensor.matmul(psqk[:], sel36_s[:], sq1[:], start=False, stop=True)
                nc.tensor.matmul(psqk[32:33, :], onesr[0:64, 0:1], sqk[:],
                                 start=False, stop=True, skip_group_check=True)

            def u_fact():
                # f = rsqrt(ssq/HD + eps) = exp(-0.5 * ln(ssq/HD + eps))
                lnm = lns.tile([33, w], F32, name="lnm", tag="lnm")
                nc.scalar.activation(lnm[:], st['psqk'][0:33, :], AF.Ln,
                                     bias=epsb[0:33, :], scale=1.0 / HD)
                ft = lns.tile([33, w], BF16, name="ft", tag="ft")
                nc.scalar.activation(ft[:], lnm[:], AF.Exp, scale=-0.5)
                if KDEBUG:
                    nc.sync.dma_start(dbg["d_ft"][:, sl], ft[:])
                fbq_ps = pprj.tile([128, w], F32, name="fbq", tag="prj")
                nc.tensor.matmul(fbq_ps[:], gsel_s[:], ft[0:4, :], start=True, stop=True)
                fbk_ps = pprj.tile([64, w], F32, name="fbk", tag="prj")
                nc.tensor.matmul(fbk_ps[:], onesr[32:33, 0:64], ft[32:33, :],
                                 start=True, stop=True, skip_group_check=True)
                fbq = lns.tile([128, w], BF16, name="fbq_s", tag="fbq_s")
                fbk = lns.tile([64, w], BF16, name="fbk_s", tag="fbk_s")
                nc.vector.tensor_copy(fbq[:], fbq_ps[:])
                nc.vector.tensor_copy(fbk[:], fbk_ps[:])
                st['fbq'], st['fbk'] = fbq, fbk

            def u_ropeq():
                fbq = st['fbq']
                t1 = rt.tile([128, w], BF16, name="t1", tag="t1")
                t2 = rt.tile([128, w], BF16, name="t2", tag="t2")
                qr0 = rt.tile([128, w], BF16, name="qr0", tag="qr0")
                qr1 = rt.tile([128, w], BF16, name="qr1", tag="qr1")
                nc.vector.tensor_mul(t1[:], qsb[0][:, sl], cs[:, sl])
                nc.vector.tensor_mul(t2[:], qsb[1][:, sl], cs[:, sls])
                nc.vector.tensor_add(t1[:], t1[:], t2[:])
                nc.vector.tensor_mul(qr0[:], t1[:], fbq[:])
                u1 = rt.tile([128, w], BF16, name="u1", tag="u1")
                u2 = rt.tile([128, w], BF16, name="u2", tag="u2")
                nc.vector.tensor_mul(u1[:], qsb[1][:, sl], cs[:, sl])
                nc.vector.tensor_mul(u2[:], qsb[0][:, sl], cs[:, sls])
                nc.vector.tensor_sub(u1[:], u1[:], u2[:])
                nc.vector.tensor_mul(qr1[:], u1[:], fbq[:])
                # reassemble: head h -> qp[h%2] rows 64*(h//2)?? no:
                # qp[0] rows 0:64 (h0 at u=0, h2 at u=1), qp[1] rows 64:128
                # (h1 at u=0, h3 at u=1); rope top half rows +0, bottom +32.
                for h in range(4):
                    dst = qp[h % 2]
                    rbase = 64 * (h % 2)
                    u = h // 2
                    hs = slice(32 * h, 32 * h + 32)
                    nc.sync.dma_start(dst[rbase:rbase + 32, u, sl], qr0[hs, :])
                    nc.sync.dma_start(dst[rbase + 32:rbase + 64, u, sl], qr1[hs, :])

            def u_ropek():
                fbk = st['fbk']
                k1 = rt.tile([32, w], BF16, name="k1", tag="k1")
                k2 = rt.tile([32, w], BF16, name="k2", tag="k2")
                kw0 = rt.tile([32, w], BF16, name="kw0", tag="kw0")
                kw1 = rt.tile([32, w], BF16, name="kw1", tag="kw1")
                nc.vector.tensor_mul(k1[:], kvsb[0:32, sl], cs[0:32, sl])
                nc.vector.tensor_mul(k2[:], kb0[:, sl], cs[0:32, sls])
                nc.vector.tensor_add(k1[:], k1[:], k2[:])
                nc.vector.tensor_mul(kw0[:], k1[:], fbk[0:32, :])
                k3 = rt.tile([32, w], BF16, name="k3", tag="k3")
                k4 = rt.tile([32, w], BF16, name="k4", tag="k4")
                nc.vector.tensor_mul(k3[:], kb0[:, sl], cs[0:32, sl])
                nc.vector.tensor_mul(k4[:], kvsb[0:32, sl], cs[0:32, sls])
                nc.vector.tensor_sub(k3[:], k3[:], k4[:])
                nc.vector.tensor_mul(kw1[:], k3[:], fbk[0:32, :])
                nc.sync.dma_start(kdup[0:32, sl], kw0[:])
                nc.sync.dma_start(kdup[32:64, sl], kw1[:])
                nc.sync.dma_start(kdup[64:96, sl], kw0[:])
                nc.sync.dma_start(kdup[96:128, sl], kw1[:])

            def u_vtr():
                ntr = w // 128
                j0 = c0 // 128
                ptr = pprj.tile([128, 64 * ntr], BF16, name="ptr", tag="prj")
                for t in range(ntr):
                    stj = j0 + t
                    nc.tensor.transpose(
                        ptr[:, 64 * t:64 * (t + 1)],
                        kvsb[64:128, 128 * stj:128 * (stj + 1)],
                        id128_s[64:128, 64:128])
                nc.vector.tensor_copy(
                    vsb[:, j0:j0 + ntr, 0:64],
                    ptr[:].rearrange("p (t e) -> p t e", t=ntr))
                for b_ in rdy_blocks:
                    ready[b_] = True

            return [u_pq0a, u_pq0b, u_pq1a, u_pq1b, u_pkva, u_pkvb,
                    u_ssq, u_fact, u_ropeq, u_ropek, u_vtr]

        # ================= attention machinery =================
        blk = {}            # b -> (yt01, yt23)
        pend_norm = [None]  # (b, ysb, rbc)
        pt_tiles = {}       # (b, j) -> pt tile
        stl_tiles = {}      # (b, j) -> pre-allocated scores psum

        def prealloc_stl(b, j):
            """Allocate the scores psum for iteration (b, j) and, on diagonal
            iterations, let the DVE write the causal mask bias into it ahead
            of the PE's score matmuls."""
            stl = pbig.tile([128, 1024], F32, name="stl", tag="big")
            diag = j - 2 * b
            if diag >= 0:
                mbsl = slice(512 * diag, 512 * (diag + 1))
                nc.vector.tensor_copy(stl[:, 0:512], mbt[:, mbsl])
                nc.vector.tensor_copy(stl[:, 512:1024], mbt[:, mbsl])
            stl_tiles[(b, j)] = stl

        def emit_sc(b, j):
            sq = slice(SQB * b, SQB * (b + 1))
            jt = slice(128 * j, 128 * (j + 1))
            stl = stl_tiles.pop((b, j))
            sflag = j - 2 * b < 0   # off-diag: no DVE mask pre-write
            # cols: h0 0:256 | h2 256:512 | h1 512:768 | h3 768:1024
            nc.tensor.matmul(stl[:, 0:512], kdup[0:64, jt],
                             qp[0][0:64, :, sq], start=sflag, stop=True,
                             skip_group_check=True)
            nc.tensor.matmul(stl[:, 512:1024], kdup[64:128, jt],
                             qp[1][64:128, :, sq], start=sflag, stop=True,
                             skip_group_check=True)
            pt = pa.tile([128, 1024], BF16, name="pt", tag="pt")
            nc.scalar.activation(pt[:], stl[:], AF.Exp, bias=e8b[:, :])
            pt_tiles[(b, j)] = pt

        def emit_den_chain(b, y1, y2):
            """Denominator recip + bf16 drain of yt, right after the final pv
            of block b.  recip runs in a [128,8] layout (shape-mismatched DMA
            shift via the scalar queue) - 8 DVE cycles instead of 1024."""
            dcp = sst.tile([65, 1024], F32, name="dcp", tag="dcp")
            nc.vector.tensor_copy(dcp[64:65, 0:512], y1[64:65, :])
            nc.vector.tensor_copy(dcp[64:65, 512:1024], y2[64:65, :])
            dlow = sst.tile([128, 8], F32, name="dlow", tag="dlow")
            nc.scalar.dma_start(dlow[:], dcp[64:65, 0:1024])
            rlow = sst.tile([128, 8], F32, name="rlow", tag="rlow")
            # tracked DVE read of dlow before the untracked custom-op read
            nc.vector.tensor_copy(rlow[:, 0:1], dlow[:, 0:1])
            nc.vector.reciprocal_approx_fast(rlow[:], dlow[:])
            rb8 = sst.tile([128, 8], F16, name="rb8", tag="rb8")
            nc.vector.tensor_copy(rb8[:], rlow[:])   # tracked sentry copy
            rbc = sst.tile([1, 1024], F16, name="rbc", tag="rbc")
            nc.scalar.dma_start(rbc[:], rb8[:])
            ysb = yb.tile([64, 1024], BF16, name="ysb", tag="ysb")
            nc.vector.tensor_copy(ysb[:, 0:512], y1[0:64, :])
            nc.vector.tensor_copy(ysb[:, 512:1024], y2[0:64, :])
            pend_norm[0] = (b, ysb, rbc)

        def make_pv(b, j):
            jmax = 2 * b + 1

            def f():
                if j == 0:
                    yt01 = py01.tile([65, 512], F32, name="yt01", tag="yt01")
                    yt23 = py23.tile([65, 512], F32, name="yt23", tag="yt23")
                    blk[b] = (yt01, yt23)
                y1, y2 = blk[b]
                pt = pt_tiles.pop((b, j))
                nc.tensor.matmul(y1[:], vsb[:, j, 0:65], pt[:, 0:512],
                                 start=(j == 0), stop=(j == jmax),
                                 skip_group_check=True)
                nc.tensor.matmul(y2[:], vsb[:, j, 0:65], pt[:, 512:1024],
                                 start=(j == 0), stop=(j == jmax),
                                 skip_group_check=True)
                if j == jmax:
                    emit_den_chain(b, y1, y2)
            return f

        def finish_normalize():
            b, ysb, rbc = pend_norm[0]
            pend_norm[0] = None
            sq = slice(SQB * b, SQB * (b + 1))
            pbb0 = pprj.tile([64, 512], F32, name="pbb0", tag="prj")
            nc.tensor.matmul(pbb0[:], onesf16[0:1, 0:64], rbc[0:1, 0:512],
                             start=True, stop=True, skip_group_check=True)
            pbb1 = pprj.tile([64, 512], F32, name="pbb1", tag="prj")
            nc.tensor.matmul(pbb1[:], onesf16[0:1, 0:64], rbc[0:1, 512:1024],
                             start=True, stop=True, skip_group_check=True)
            nc.vector.tensor_copy(pbs[:, 0:512], pbb0[:])
            nc.vector.tensor_copy(pbs[:, 512:1024], pbb1[:])
            nc.vector.tensor_mul(
                ynA[0:64, :, sq],
                ysb[:, 0:512].rearrange("p (u c) -> p u c", u=2),
                pbs[:, 0:512].rearrange("p (u c) -> p u c", u=2))
            sg1 = sst.tile([64, 2, 256], BF16, name="sg1", tag="sg1")
            nc.vector.tensor_mul(
                sg1[:],
                ysb[:, 512:1024].rearrange("p (u c) -> p u c", u=2),
                pbs[:, 512:1024].rearrange("p (u c) -> p u c", u=2))
            nc.sync.dma_start(ynA[64:128, :, sq], sg1[:])
            if KDEBUG and b == 0:
                nc.sync.dma_start(dbg["d_pbs"][:], pbs[:])
                nc.sync.dma_start(dbg["d_ysb"][:], ysb[:])
                nc.sync.dma_start(dbg["d_rbc"][:], rbc[:])
            return b

        def wo_units(b):
            units = []
            for t2 in range(2):
                sti = 2 * b + t2
                ssl = slice(128 * sti, 128 * (sti + 1))
                stt = {}

                def ua(ssl=ssl, stt=stt):
                    pot0 = pprj.tile([128, 512], F32, name="pot0", tag="prj")
                    stt['pot0'] = pot0
                    nc.tensor.matmul(pot0[:], ynA[:, 0, ssl], wo[:, 0, 0:512],
                                     start=True, stop=False)
                    nc.tensor.matmul(pot0[:], ynA[:, 1, ssl], wo[:, 1, 0:512],
                                     start=False, stop=True)

                def ub(ssl=ssl, stt=stt):
                    pot1 = pprj.tile([128, 512], F32, name="pot1", tag="prj")
                    stt['pot1'] = pot1
                    ot = ob.tile([128, D], F16, name="ot", tag="ot")
                    stt['ot'] = ot
                    nc.tensor.matmul(pot1[:], ynA[:, 0, ssl], wo[:, 0, 512:1024],
                                     start=True, stop=False)
                    nc.tensor.matmul(pot1[:], ynA[:, 1, ssl], wo[:, 1, 512:1024],
                                     start=False, stop=True)
                    nc.scalar.copy(ot[:, 0:512], stt['pot0'][:])

                def uc(ssl=ssl, stt=stt):
                    ot = stt['ot']
                    nc.vector.tensor_copy(ot[:, 512:1024], stt['pot1'][:])
                    nc.sync.dma_start(out_d[ssl, :], ot[:])

                units += [ua, ub, uc]
            return units

        # ================= main schedule =================
        for u in chunk_units(0, [0, 1]):
            u()

        wo_backlog = deque()
        prev_pv = None
        for b in range(NB):
            pump_until_ready(b)
            if b == 0:
                for u in chunk_units(512, [2, 3]):
                    fq_hi.append(u)
            elif b == 2:
                for u in chunk_units(1024, [4, 5]):
                    fq_hi.append(u)
            elif b == 4:
                for u in chunk_units(1536, [6, 7]):
                    fq_hi.append(u)
            while wo_backlog:
                fq_lo.append(wo_backlog.popleft())
            jmax = 2 * b + 1
            inject_at = min(4, jmax)
            prealloc_stl(b, 0)
            for j in range(jmax + 1):
                emit_sc(b, j)
                # pre-allocate the next iteration's scores psum (mask goes in
                # via DVE while this iteration computes)
                if j < jmax:
                    prealloc_stl(b, j + 1)
                elif b + 1 < NB:
                    prealloc_stl(b + 1, 0)
                if prev_pv is not None:
                    prev_pv()
                prev_pv = make_pv(b, j)
                if j == inject_at and pend_norm[0] is not None:
                    nb_ = finish_normalize()
                    units = wo_units(nb_)
                    if b == NB - 1:
                        for u in units:
                            fq_lo.append(u)
                    else:
                        wo_backlog.extend(units)
                pump(2)

        # ---- tail ----
        prev_pv()                 # final pv + den chain for block 7
        while fq_hi:
            fq_hi.popleft()()
        while fq_lo:
            fq_lo.popleft()()
        finish_normalize()        # block 7
        for u in wo_units(NB - 1):
            u()
        if KDEBUG:
            nc.sync.dma_start(dbg["d_qsb0"][:], qsb[0][:])
            nc.sync.dma_start(dbg["d_qsb1"][:], qsb[1][:])
            nc.sync.dma_start(dbg["d_kvsb"][:], kvsb[:])
            nc.sync.dma_start(dbg["d_kdup"][:], kdup[:])
            nc.sync.dma_start(dbg["d_vsb"][:], vsb[:])
            nc.sync.dma_start(dbg["d_yn0"][:], ynA[:, 0, :])
            nc.sync.dma_start(dbg["d_yn1"][:], ynA[:, 1, :])

    nc.finalize()
    return nc


_NC = None


def _get_nc():
    global _NC
    if _NC is None:
        _NC = _build()
    return _NC


def _perm():
    tops = [h * 64 + i for h in range(HG) for i in range(32)]
    bots = [h * 64 + 32 + i for h in range(HG) for i in range(32)]
    return tops + bots


def build_inmaps(x, Wq, Wk, Wv, Wo, q_gain):
    x = np.asarray(x, dtype=np.float32)
    Wq = np.asarray(Wq, dtype=np.float32)
    Wk = np.asarray(Wk, dtype=np.float32)
    Wv = np.asarray(Wv, dtype=np.float32)
    Wo = np.asarray(Wo, dtype=np.float32)
    q_gain = np.asarray(q_gain, dtype=np.float32)

    perm = _perm()
    xTs = [np.ascontiguousarray(x[dp].T).astype(BF16NP) for dp in range(2)]
    tp_maps = []
    for tp in range(4):
        wq_sel = Wq[tp * E:(tp + 1) * E].T[:, perm]          # [D, 256] permuted
        wq_t = np.ascontiguousarray(
            wq_sel.astype(BF16NP).reshape(NK, 128, E).transpose(1, 0, 2))
        wk_sel = Wk[tp * HD:(tp + 1) * HD].T                  # [D, 64]
        wv_sel = Wv[tp * HD:(tp + 1) * HD].T
        wkv_t = np.concatenate([wk_sel, wv_sel], axis=1).astype(BF16NP)
        wkv_t = np.ascontiguousarray(
            wkv_t.reshape(NK, 128, 128).transpose(1, 0, 2))
        wo_sel = Wo[:, tp * E:(tp + 1) * E].T                 # [256, D]
        wo_t = np.ascontiguousarray(
            wo_sel.astype(BF16NP).reshape(2, 128, D).transpose(1, 0, 2))
        g = q_gain[tp * HG:(tp + 1) * HG].astype(np.float32)
        gsel = np.zeros((4, 128), dtype=BF16NP)
        for h in range(4):
            gsel[h, 32 * h:32 * h + 32] = BF16NP(g[h] / 8.0)
        tp_maps.append({"wq": wq_t, "wkv": wkv_t, "wo": wo_t, "gsel": gsel})
    in_maps = []
    for c in range(8):
        dp, tp = divmod(c, 4)
        m = dict(tp_maps[tp])
        m["xT"] = xTs[dp]
        in_maps.append(m)
    return in_maps


def kernel(x, Wq, Wk, Wv, Wo, q_gain):
    in_maps = build_inmaps(x, Wq, Wk, Wv, Wo, q_gain)
    nc = _get_nc()
    res = run_bass_kernel_spmd(nc, in_maps, core_ids=list(range(8)))
    out = np.zeros((B, S, D), dtype=np.float32)
    for c in range(8):
        out[c // 4] += res.results[c]["out"].astype(np.float32)
    return out
